# revision 3
# baseline (speedup 1.0000x reference)
"""Trainium2 Bass kernel for ParallelLMHeadWithLoRA.

out[t, v] = hidden[t] @ weight[v]^T + xa[t] @ lora_B[e_t, v]^T
            where xa[t] = hidden[t] @ lora_A[e_t]^T,  e_t = weight_indices[t]

Sharding: column-parallel on vocab across 8 cores — weight and lora_B are
sharded along V (4000 rows/core), hidden / lora_A / weight_indices are
replicated.  Each core computes out[:, shard]; the host concatenates.

Device algorithm (per core), all fp32 data with fp32r matmuls (full-rate
PE, ~1e-4 rel err):
  - Phase W: PE-transpose the weight shard once into a DRAM scratch WTs
    [32 k-tiles, 128 h, 4000 v] (fp32r-rounded).
  - Phase A/B: lora_A^T -> ATs scratch, lora_B^T -> BTs scratch.
  - Two token passes of 1024: transpose hidden block into SBUF-resident
    hT; compute xa^T = A_all^T-blocks @ hT; mask by expert (one-hot of
    weight_indices); then per 512-wide vocab panel accumulate 32 base
    matmuls + 1 lora matmul into PSUM (8 banks = 8 token tiles) and
    store.
"""

import numpy as np

T = 2048
H = 4096
V = 32000
NCORES = 8
VS = V // NCORES  # 4000
E = 8
R = 16
P = 128
KT = H // P  # 32
TB = 1024  # tokens per pass
NPASS = T // TB
PANELS = [(i * 512, 512) for i in range(7)] + [(3584, VS - 3584)]  # 7x512 + 416

_CACHE = {}


def _build_nc():
    from concourse import bacc
    import concourse.mybir as mybir
    from concourse.tile import TileContext
    from concourse.masks import make_identity

    f32 = mybir.dt.float32
    f32r = mybir.dt.float32r
    i32 = mybir.dt.int32

    nc = bacc.Bacc("TRN2", target_bir_lowering=False, debug=False)

    hid_d = nc.dram_tensor("hidden", [T, H], f32, kind="ExternalInput")
    w_d = nc.dram_tensor("weight", [VS, H], f32, kind="ExternalInput")
    la_d = nc.dram_tensor("lora_a", [P, H], f32, kind="ExternalInput")  # [E*R, H]
    lb_d = nc.dram_tensor("lora_b", [E, VS, R], f32, kind="ExternalInput")
    idx_d = nc.dram_tensor("widx", [1, T], i32, kind="ExternalInput")
    out_d = nc.dram_tensor("out", [T, VS], f32, kind="ExternalOutput")

    wts_d = nc.dram_tensor("wts", [KT, P, VS], f32r, kind="Internal")
    ats_d = nc.dram_tensor("ats", [KT, P, P], f32r, kind="Internal")
    bts_d = nc.dram_tensor("bts", [P, VS], f32r, kind="Internal")

    # v-blocks of the shard: 31x128 + 32
    VBLOCKS = [(i * P, P) for i in range(VS // P)]
    if VS % P:
        VBLOCKS.append((VS - VS % P, VS % P))

    with TileContext(nc) as tc:
        ident, free_ident = tc.tile([P, P], f32, name="ident")
        make_identity(nc, ident)
        hT, free_hT = tc.tile([P, KT * TB], f32r, name="hT")
        hT_k = hT.rearrange("p (k t) -> p k t", t=TB)

        with (
            tc.tile_pool(name="psp", bufs=8, space="PSUM") as psp,
            tc.tile_pool(name="natp", bufs=4) as natp,      # [128,512] f32 staging
            tc.tile_pool(name="wstp", bufs=3) as wstp,      # [128,512] f32r store staging
            tc.tile_pool(name="nathp", bufs=2) as nathp,    # [128,2048] f32 staging
            tc.tile_pool(name="wldp", bufs=4) as wldp,      # [128,512] f32r loads
            tc.tile_pool(name="atldp", bufs=2) as atldp,    # [128,128] f32r loads
            tc.tile_pool(name="btldp", bufs=2) as btldp,    # [128,512] f32r loads
            tc.tile_pool(name="ostp", bufs=4) as ostp,      # [128,512] f32 out staging
            tc.tile_pool(name="maskp", bufs=1) as maskp,
            tc.tile_pool(name="btstp", bufs=2) as btstp,    # [16,512] f32r staging
        ):
            # ---- constants for mask ----
            p_col_i = maskp.tile([P, 1], i32, tag="pci")
            nc.gpsimd.iota(p_col_i, pattern=[[0, 1]], base=0, channel_multiplier=1)
            p_col = maskp.tile([P, 1], f32, tag="pcf")
            nc.vector.tensor_copy(p_col, p_col_i)

            # ---- lora_A^T -> ATs ----
            for half in range(2):
                nat_a = nathp.tile([P, 2048], f32, tag="nath")
                nc.sync.dma_start(nat_a, la_d[:, half * 2048:(half + 1) * 2048])
                for ks in range(4):
                    ps = psp.tile([P, 512], f32, tag="bank")
                    for kk in range(4):
                        nc.tensor.transpose(
                            ps[:, kk * P:(kk + 1) * P],
                            nat_a[:, (ks * 4 + kk) * P:(ks * 4 + kk + 1) * P],
                            ident,
                        )
                    st = wstp.tile([P, 512], f32r, tag="wst")
                    nc.vector.tensor_copy(st, ps)
                    k0 = half * 16 + ks * 4
                    nc.sync.dma_start(
                        ats_d[k0:k0 + 4, :, :].rearrange("k h e -> h k e"), st
                    )

            # ---- lora_B^T -> BTs ----
            NB_FULL = VS // P  # 31
            REM = VS % P       # 32
            for e in range(E):
                nat_b = natp.tile([P, 512], f32, tag="nat")
                nc.sync.dma_start(
                    nat_b[:, 0:NB_FULL * R],
                    lb_d[e, 0:NB_FULL * P, :].rearrange("(vt v) r -> v vt r", v=P),
                )
                nc.sync.dma_start(
                    nat_b[0:REM, NB_FULL * R:NB_FULL * R + R],
                    lb_d[e, NB_FULL * P:VS, :],
                )
                for grp in range(8):
                    vts = [grp * 4 + j for j in range(4)]
                    ps = psp.tile([P, 512], f32, tag="bank")
                    off = 0
                    widths = []
                    for vt in vts:
                        vsz = P if vt < NB_FULL else REM
                        nc.tensor.transpose(
                            ps[0:R, off:off + vsz],
                            nat_b[0:vsz, vt * R:(vt + 1) * R],
                            ident[0:vsz, 0:vsz],
                        )
                        widths.append(vsz)
                        off += vsz
                    st = btstp.tile([R, 512], f32r, tag="btst")
                    nc.vector.tensor_copy(st[:, 0:off], ps[0:R, 0:off])
                    v0 = vts[0] * P
                    nc.sync.dma_start(
                        bts_d[e * R:(e + 1) * R, v0:v0 + off], st[:, 0:off]
                    )

            # ---- Phase W: weight shard -> WTs (transposed, rounded) ----
            for (v0, vsz) in VBLOCKS:
                for ks in range(8):
                    nat_w = natp.tile([P, 512], f32, tag="nat")
                    nc.sync.dma_start(
                        nat_w[0:vsz, :], w_d[v0:v0 + vsz, ks * 512:(ks + 1) * 512]
                    )
                    ps = psp.tile([P, 512], f32, tag="bank")
                    for kk in range(4):
                        nc.tensor.transpose(
                            ps[:, kk * vsz:(kk + 1) * vsz],
                            nat_w[0:vsz, kk * P:(kk + 1) * P],
                            ident[0:vsz, 0:vsz],
                        )
                    st = wstp.tile([P, 512], f32r, tag="wst")
                    nc.vector.tensor_copy(st[:, 0:4 * vsz], ps[:, 0:4 * vsz])
                    nc.sync.dma_start(
                        wts_d[ks * 4:(ks + 1) * 4, :, v0:v0 + vsz].rearrange(
                            "k h v -> h k v"
                        ),
                        st[:, 0:4 * vsz],
                    )

            # ---- token passes ----
            for pz in range(NPASS):
                t0 = pz * TB

                # hT build: transpose hidden[t0:t0+TB]
                for t8 in range(TB // P):
                    for half in range(2):
                        nat_h = nathp.tile([P, 2048], f32, tag="nath")
                        nc.sync.dma_start(
                            nat_h,
                            hid_d[
                                t0 + t8 * P:t0 + (t8 + 1) * P,
                                half * 2048:(half + 1) * 2048,
                            ],
                        )
                        for ks in range(4):
                            ps = psp.tile([P, 512], f32, tag="bank")
                            for kk in range(4):
                                nc.tensor.transpose(
                                    ps[:, kk * P:(kk + 1) * P],
                                    nat_h[:, (ks * 4 + kk) * P:(ks * 4 + kk + 1) * P],
                                    ident,
                                )
                            k0 = half * 16 + ks * 4
                            nc.vector.tensor_copy(
                                hT_k[:, k0:k0 + 4, t8 * P:(t8 + 1) * P], ps
                            )

                # mask for this pass: mask[p, t] = (widx[t] == p//16)
                idxp = maskp.tile([1, TB], i32, tag="idxp")
                nc.sync.dma_start(idxp, idx_d[:, t0:t0 + TB])
                idx_f = maskp.tile([1, TB], f32, tag="idxf")
                nc.vector.tensor_copy(idx_f, idxp)
                nc.vector.tensor_scalar_mul(idx_f, idx_f, 16.0)
                bc = maskp.tile([P, TB], f32, tag="bc")
                nc.gpsimd.partition_broadcast(bc, idx_f)
                d = maskp.tile([P, TB], f32, tag="d")
                nc.vector.tensor_scalar(
                    d, bc, p_col, None, mybir.AluOpType.subtract
                )
                u1 = maskp.tile([P, TB], f32, tag="u1")
                nc.vector.tensor_scalar(u1, d, 0.0, None, mybir.AluOpType.is_le)
                nc.vector.tensor_scalar(d, d, -15.0, None, mybir.AluOpType.is_ge)
                mask = bc  # reuse bc slot: mask = u1 * d
                nc.vector.tensor_tensor(mask, u1, d, mybir.AluOpType.mult)

                # xa^T for this pass: [128 er, TB]
                mxaT = maskp.tile([P, TB], f32r, tag="mxa")
                for g in range(TB // 512):
                    xa_ps = psp.tile([P, 512], f32, tag="bank")
                    for k in range(KT):
                        atk = atldp.tile([P, P], f32r, tag="atld")
                        nc.sync.dma_start(atk, ats_d[k, :, :])
                        nc.tensor.matmul(
                            xa_ps,
                            atk,
                            hT_k[:, k, g * 512:(g + 1) * 512],
                            start=(k == 0),
                            stop=(k == KT - 1),
                        )
                    nc.vector.tensor_tensor(
                        mxaT[:, g * 512:(g + 1) * 512],
                        xa_ps,
                        mask[:, g * 512:(g + 1) * 512],
                        mybir.AluOpType.mult,
                    )

                # vocab panels
                for (v0, np_) in PANELS:
                    accs = [
                        psp.tile([P, 512], f32, tag="bank", name=f"acc{pz}_{v0}_{i}")
                        for i in range(TB // P)
                    ]
                    for k in range(KT):
                        wtk = wldp.tile([P, 512], f32r, tag="wld")
                        nc.sync.dma_start(wtk[:, 0:np_], wts_d[k, :, v0:v0 + np_])
                        for t8 in range(TB // P):
                            nc.tensor.matmul(
                                accs[t8][:, 0:np_],
                                hT_k[:, k, t8 * P:(t8 + 1) * P],
                                wtk[:, 0:np_],
                                start=(k == 0),
                                stop=False,
                            )
                    btk = btldp.tile([P, 512], f32r, tag="btld")
                    nc.sync.dma_start(btk[:, 0:np_], bts_d[:, v0:v0 + np_])
                    for t8 in range(TB // P):
                        nc.tensor.matmul(
                            accs[t8][:, 0:np_],
                            mxaT[:, t8 * P:(t8 + 1) * P],
                            btk[:, 0:np_],
                            start=False,
                            stop=True,
                        )
                    for t8 in range(TB // P):
                        o_sb = ostp.tile([P, 512], f32, tag="ost")
                        nc.scalar.copy(o_sb[:, 0:np_], accs[t8][:, 0:np_])
                        nc.sync.dma_start(
                            out_d[t0 + t8 * P:t0 + (t8 + 1) * P, v0:v0 + np_],
                            o_sb[:, 0:np_],
                        )

        free_hT()
        free_ident()

    nc.finalize()
    return nc


def _get_nc():
    if "nc" not in _CACHE:
        _CACHE["nc"] = _build_nc()
    return _CACHE["nc"]


def run_sharded(inputs, trace=False):
    from concourse import bass_utils

    hidden = np.ascontiguousarray(inputs["hidden_states"], dtype=np.float32)
    weight = np.ascontiguousarray(inputs["weight"], dtype=np.float32)
    lora_A = np.ascontiguousarray(inputs["lora_A"], dtype=np.float32).reshape(E * R, H)
    lora_B = np.ascontiguousarray(inputs["lora_B"], dtype=np.float32)
    widx = np.ascontiguousarray(inputs["weight_indices"], dtype=np.int32).reshape(1, T)

    nc = _get_nc()
    in_maps = []
    for c in range(NCORES):
        in_maps.append(
            {
                "hidden": hidden,
                "weight": weight[c * VS:(c + 1) * VS],
                "lora_a": lora_A,
                "lora_b": lora_B[:, c * VS:(c + 1) * VS, :],
                "widx": widx,
            }
        )
    res = bass_utils.run_bass_kernel_spmd(
        nc, in_maps, core_ids=list(range(NCORES)), trace=trace
    )
    out = np.concatenate([res.results[c]["out"] for c in range(NCORES)], axis=1)
    return out, res


def kernel(**inputs) -> np.ndarray:
    out, _ = run_sharded(inputs, trace=False)
    return out


# mybir import is needed at build time inside _build_nc's closure
import concourse.mybir as mybir  # noqa: E402


# revision 6
# speedup vs baseline: 1.1332x; 1.1332x over previous
"""Trainium2 Bass kernel for ParallelLMHeadWithLoRA.

out[t, v] = hidden[t] @ weight[v]^T + xa[t] @ lora_B[e_t, v]^T
            where xa[t] = hidden[t] @ lora_A[e_t]^T,  e_t = weight_indices[t]

Sharding: column-parallel on vocab across 8 cores — weight and lora_B are
sharded along V (4000 rows/core), hidden / lora_A / weight_indices are
replicated.  Each core computes out[:, shard]; the host concatenates.

Per-core schedule (all fp32 data, fp32r matmuls):
  - consts, lora_A^T -> ATs scratch, lora_B^T -> BTs scratch
  - hT build for tokens 0:1024 (PE transposes, SBUF-resident)
  - xa^T + expert mask for tokens 0:1024
  - sub-pass A1 (tokens 0:512): FUSED weight transpose — per 512-wide
    vocab panel and k-tile, PE-transpose the natural weight block,
    matmul it immediately (4 PSUM banks accumulate, 3 cycle transposes),
    and store the transposed panel to WTs DRAM scratch for later passes.
  - sub-pass A2 (tokens 512:1024): streams WTs, 4 PSUM banks.
  - hT rebuild + xa/mask for tokens 1024:2048
  - pass B (tokens 1024:2048): streams WTs, 8 PSUM banks.
"""

import numpy as np

T = 2048
H = 4096
V = 32000
NCORES = 8
VS = V // NCORES  # 4000
E = 8
R = 16
P = 128
KT = H // P  # 32
TB = 1024  # tokens per hT residency
PANELS = [(i * 512, 512) for i in range(7)] + [(3584, VS - 3584)]  # 7x512 + 416

_CACHE = {}


def _build_nc():
    from concourse import bacc
    import concourse.mybir as mybir
    from concourse.tile import TileContext
    from concourse.masks import make_identity

    f32 = mybir.dt.float32
    f32r = mybir.dt.float32r
    bf16 = mybir.dt.bfloat16
    i32 = mybir.dt.int32
    OP = mybir.AluOpType

    nc = bacc.Bacc("TRN2", target_bir_lowering=False, debug=False)

    hid_d = nc.dram_tensor("hidden", [T, H], f32, kind="ExternalInput")
    w_d = nc.dram_tensor("weight", [VS, H], f32, kind="ExternalInput")
    la_d = nc.dram_tensor("lora_a", [P, H], f32, kind="ExternalInput")  # [E*R, H]
    lb_d = nc.dram_tensor("lora_b", [E, VS, R], f32, kind="ExternalInput")
    idx_d = nc.dram_tensor("widx", [1, T], i32, kind="ExternalInput")
    out_d = nc.dram_tensor("out", [T, VS], f32, kind="ExternalOutput")

    wts_d = nc.dram_tensor("wts", [KT, P, VS], f32r, kind="Internal")
    ats_d = nc.dram_tensor("ats", [KT, P, P], f32r, kind="Internal")
    bts_d = nc.dram_tensor("bts", [P, VS], f32r, kind="Internal")

    with TileContext(nc) as tc:
        ident, free_ident = tc.tile([P, P], f32, name="ident")
        make_identity(nc, ident)
        hT, free_hT = tc.tile([P, KT * TB], f32r, name="hT")
        hT_k = hT.rearrange("p (k t) -> p k t", t=TB)

        with (
            tc.tile_pool(name="psp", bufs=8, space="PSUM") as psp,
            tc.tile_pool(name="natp", bufs=8) as natp,      # [128,512] f32 staging
            tc.tile_pool(name="wstp", bufs=3) as wstp,      # [128,512] f32r wT tiles
            tc.tile_pool(name="nathp", bufs=2) as nathp,    # [128,2048] f32 staging
            tc.tile_pool(name="wldp", bufs=2) as wldp,      # [128,1024] f32r loads
            tc.tile_pool(name="atldp", bufs=2) as atldp,    # [128,128] f32r loads
            tc.tile_pool(name="btldp", bufs=2) as btldp,    # [128,512] f32r loads
            tc.tile_pool(name="ostp", bufs=3) as ostp,      # [128,512] f32 out staging
            tc.tile_pool(name="maskp", bufs=1) as maskp,
            tc.tile_pool(name="btstp", bufs=1) as btstp,    # [16,512] f32r staging
        ):
            # ---- constants for mask ----
            p_col_i = maskp.tile([P, 1], i32, tag="pci")
            nc.gpsimd.iota(p_col_i, pattern=[[0, 1]], base=0, channel_multiplier=1)
            p_col = maskp.tile([P, 1], f32, tag="pcf")
            nc.vector.tensor_copy(p_col, p_col_i)

            # ---- lora_A^T -> ATs ----
            for half in range(2):
                nat_a = nathp.tile([P, 2048], f32, tag="nath")
                nc.sync.dma_start(nat_a, la_d[:, half * 2048:(half + 1) * 2048])
                for ks in range(4):
                    ps = psp.tile([P, 512], f32, tag="bank")
                    for kk in range(4):
                        nc.tensor.transpose(
                            ps[:, kk * P:(kk + 1) * P],
                            nat_a[:, (ks * 4 + kk) * P:(ks * 4 + kk + 1) * P],
                            ident,
                        )
                    st = wstp.tile([P, 512], f32r, tag="wst")
                    nc.vector.tensor_copy(st, ps)
                    k0 = half * 16 + ks * 4
                    nc.sync.dma_start(
                        ats_d[k0:k0 + 4, :, :].rearrange("k h e -> h k e"), st
                    )

            # ---- lora_B^T -> BTs ----
            NB_FULL = VS // P  # 31
            REM = VS % P       # 32
            for e in range(E):
                nat_b = natp.tile([P, 512], f32, tag="nat")
                nc.sync.dma_start(
                    nat_b[:, 0:NB_FULL * R],
                    lb_d[e, 0:NB_FULL * P, :].rearrange("(vt v) r -> v vt r", v=P),
                )
                nc.sync.dma_start(
                    nat_b[0:REM, NB_FULL * R:NB_FULL * R + R],
                    lb_d[e, NB_FULL * P:VS, :],
                )
                for grp in range(8):
                    vts = [grp * 4 + j for j in range(4)]
                    ps = psp.tile([P, 512], f32, tag="bank")
                    off = 0
                    for vt in vts:
                        vsz = P if vt < NB_FULL else REM
                        nc.tensor.transpose(
                            ps[0:R, off:off + vsz],
                            nat_b[0:vsz, vt * R:(vt + 1) * R],
                            ident[0:vsz, 0:vsz],
                        )
                        off += vsz
                    st = btstp.tile([R, 512], f32r, tag="btst")
                    nc.vector.tensor_copy(st[:, 0:off], ps[0:R, 0:off])
                    v0 = vts[0] * P
                    nc.sync.dma_start(
                        bts_d[e * R:(e + 1) * R, v0:v0 + off], st[:, 0:off]
                    )

            def build_hT(t0):
                """Transpose hidden[t0:t0+TB] into the resident hT."""
                for t8 in range(TB // P):
                    for half in range(2):
                        nat_h = nathp.tile([P, 2048], f32, tag="nath")
                        nc.sync.dma_start(
                            nat_h,
                            hid_d[
                                t0 + t8 * P:t0 + (t8 + 1) * P,
                                half * 2048:(half + 1) * 2048,
                            ],
                        )
                        for ks in range(4):
                            ps = psp.tile([P, 512], f32, tag="bank")
                            for kk in range(4):
                                nc.tensor.transpose(
                                    ps[:, kk * P:(kk + 1) * P],
                                    nat_h[:, (ks * 4 + kk) * P:(ks * 4 + kk + 1) * P],
                                    ident,
                                )
                            k0 = half * 16 + ks * 4
                            nc.vector.tensor_copy(
                                hT_k[:, k0:k0 + 4, t8 * P:(t8 + 1) * P], ps
                            )

            def build_mask_xa(t0, mxaT):
                """mask[p,t] = (widx[t] == p//16); mxaT = (A_all^T@hT) * mask."""
                idxp = maskp.tile([1, TB], i32, tag="idxp")
                nc.sync.dma_start(idxp, idx_d[:, t0:t0 + TB])
                idx_f = maskp.tile([1, TB], f32, tag="idxf")
                nc.vector.tensor_copy(idx_f, idxp)
                idx16 = maskp.tile([1, TB], bf16, tag="idx16")
                nc.vector.tensor_scalar_mul(idx16, idx_f, 16.0)
                bc = maskp.tile([P, TB], bf16, tag="bc")
                nc.gpsimd.partition_broadcast(bc, idx16)
                d = maskp.tile([P, TB], bf16, tag="d")
                nc.vector.tensor_scalar(d, bc, p_col, None, OP.subtract)
                u1 = maskp.tile([P, TB], bf16, tag="u1")
                nc.vector.tensor_scalar(u1, d, 0.0, None, OP.is_le)
                nc.vector.tensor_scalar(d, d, -15.0, None, OP.is_ge)
                mask = bc  # reuse slot: mask = u1 * d
                nc.vector.tensor_tensor(mask, u1, d, OP.mult)

                for g in range(TB // 512):
                    xa_ps = psp.tile([P, 512], f32, tag="bank")
                    for k in range(KT):
                        atk = atldp.tile([P, P], f32r, tag="atld")
                        nc.scalar.dma_start(atk, ats_d[k, :, :])
                        nc.tensor.matmul(
                            xa_ps,
                            atk,
                            hT_k[:, k, g * 512:(g + 1) * 512],
                            start=(k == 0),
                            stop=(k == KT - 1),
                        )
                    nc.vector.tensor_tensor(
                        mxaT[:, g * 512:(g + 1) * 512],
                        xa_ps,
                        mask[:, g * 512:(g + 1) * 512],
                        OP.mult,
                    )

            def finish_panel(accs, mxaT, tloc0, t0, v0, np_):
                """lora matmul + copy-out + store for one panel."""
                btk = btldp.tile([P, 512], f32r, tag="btld")
                nc.scalar.dma_start(btk[:, 0:np_], bts_d[:, v0:v0 + np_])
                nt = len(accs)
                for i in range(nt):
                    nc.tensor.matmul(
                        accs[i][:, 0:np_],
                        mxaT[:, tloc0 + i * P:tloc0 + (i + 1) * P],
                        btk[:, 0:np_],
                        start=False,
                        stop=True,
                    )
                for i in range(nt):
                    o_sb = ostp.tile([P, 512], f32, tag="ost")
                    nc.vector.tensor_copy(o_sb[:, 0:np_], accs[i][:, 0:np_])
                    nc.scalar.dma_start(
                        out_d[t0 + i * P:t0 + (i + 1) * P, v0:v0 + np_],
                        o_sb[:, 0:np_],
                    )

            # ================= tokens 0:1024 =================
            build_hT(0)
            mxaT_a = maskp.tile([P, TB], f32r, tag="mxa")
            build_mask_xa(0, mxaT_a)

            # ---- A1 (tokens 0:512): fused W transpose + matmul + WTs store ----
            for (v0, np_) in PANELS:
                vbs = []  # (voff_local, vsz)
                off = 0
                while off < np_:
                    vsz = min(P, np_ - off)
                    vbs.append((off, vsz))
                    off += vsz
                accs = [
                    psp.tile([P, 512], f32, tag="bank", name=f"a1_{v0}_{i}")
                    for i in range(4)
                ]
                for ks in range(8):
                    nats = []
                    for (vo, vsz) in vbs:
                        nat_w = natp.tile([P, 512], f32, tag="nat")
                        nc.sync.dma_start(
                            nat_w[0:vsz, :],
                            w_d[v0 + vo:v0 + vo + vsz, ks * 512:(ks + 1) * 512],
                        )
                        nats.append(nat_w)
                    for kk in range(4):
                        k = ks * 4 + kk
                        ps = psp.tile([P, 512], f32, tag="bank")
                        for (vo, vsz), nat_w in zip(vbs, nats):
                            nc.tensor.transpose(
                                ps[:, vo:vo + vsz],
                                nat_w[0:vsz, kk * P:(kk + 1) * P],
                                ident[0:vsz, 0:vsz],
                            )
                        wst = wstp.tile([P, 512], f32r, tag="wst")
                        nc.vector.tensor_copy(wst[:, 0:np_], ps[:, 0:np_])
                        nc.sync.dma_start(
                            wts_d[k, :, v0:v0 + np_], wst[:, 0:np_]
                        )
                        for i in range(4):
                            nc.tensor.matmul(
                                accs[i][:, 0:np_],
                                hT_k[:, k, i * P:(i + 1) * P],
                                wst[:, 0:np_],
                                start=(k == 0),
                                stop=False,
                            )
                finish_panel(accs, mxaT_a, 0, 0, v0, np_)

            # ---- A2 (tokens 512:1024): stream WTs ----
            for (v0, np_) in PANELS:
                accs = [
                    psp.tile([P, 512], f32, tag="bank", name=f"a2_{v0}_{i}")
                    for i in range(4)
                ]
                for kh in range(KT // 2):
                    wld = wldp.tile([P, 1024], f32r, tag="wld")
                    nc.scalar.dma_start(
                        wld.rearrange("p (k v) -> p k v", v=512)[:, :, 0:np_],
                        wts_d[2 * kh:2 * kh + 2, :, v0:v0 + np_].rearrange(
                            "k h v -> h k v"
                        ),
                    )
                    for kk in range(2):
                        k = 2 * kh + kk
                        for i in range(4):
                            nc.tensor.matmul(
                                accs[i][:, 0:np_],
                                hT_k[:, k, 512 + i * P:512 + (i + 1) * P],
                                wld[:, kk * 512:kk * 512 + np_],
                                start=(k == 0),
                                stop=False,
                            )
                finish_panel(accs, mxaT_a, 512, 512, v0, np_)

            # ================= tokens 1024:2048 =================
            build_hT(TB)
            mxaT_b = maskp.tile([P, TB], f32r, tag="mxa", name="mxa_b")
            build_mask_xa(TB, mxaT_b)

            for (v0, np_) in PANELS:
                accs = [
                    psp.tile([P, 512], f32, tag="bank", name=f"b_{v0}_{i}")
                    for i in range(8)
                ]
                for kh in range(KT // 2):
                    wld = wldp.tile([P, 1024], f32r, tag="wld")
                    nc.scalar.dma_start(
                        wld.rearrange("p (k v) -> p k v", v=512)[:, :, 0:np_],
                        wts_d[2 * kh:2 * kh + 2, :, v0:v0 + np_].rearrange(
                            "k h v -> h k v"
                        ),
                    )
                    for kk in range(2):
                        k = 2 * kh + kk
                        for i in range(8):
                            nc.tensor.matmul(
                                accs[i][:, 0:np_],
                                hT_k[:, k, i * P:(i + 1) * P],
                                wld[:, kk * 512:kk * 512 + np_],
                                start=(k == 0),
                                stop=False,
                            )
                finish_panel(accs, mxaT_b, 0, TB, v0, np_)

        free_hT()
        free_ident()

    nc.finalize()
    return nc


def _get_nc():
    if "nc" not in _CACHE:
        _CACHE["nc"] = _build_nc()
    return _CACHE["nc"]


def run_sharded(inputs, trace=False):
    from concourse import bass_utils

    hidden = np.ascontiguousarray(inputs["hidden_states"], dtype=np.float32)
    weight = np.ascontiguousarray(inputs["weight"], dtype=np.float32)
    lora_A = np.ascontiguousarray(inputs["lora_A"], dtype=np.float32).reshape(E * R, H)
    lora_B = np.ascontiguousarray(inputs["lora_B"], dtype=np.float32)
    widx = np.ascontiguousarray(inputs["weight_indices"], dtype=np.int32).reshape(1, T)

    nc = _get_nc()
    in_maps = []
    for c in range(NCORES):
        in_maps.append(
            {
                "hidden": hidden,
                "weight": weight[c * VS:(c + 1) * VS],
                "lora_a": lora_A,
                "lora_b": lora_B[:, c * VS:(c + 1) * VS, :],
                "widx": widx,
            }
        )
    res = bass_utils.run_bass_kernel_spmd(
        nc, in_maps, core_ids=list(range(NCORES)), trace=trace
    )
    out = np.concatenate([res.results[c]["out"] for c in range(NCORES)], axis=1)
    return out, res


def kernel(**inputs) -> np.ndarray:
    out, _ = run_sharded(inputs, trace=False)
    return out


# revision 7
# speedup vs baseline: 1.2504x; 1.1034x over previous
"""Trainium2 Bass kernel for ParallelLMHeadWithLoRA.

out[t, v] = hidden[t] @ weight[v]^T + xa[t] @ lora_B[e_t, v]^T
            where xa[t] = hidden[t] @ lora_A[e_t]^T,  e_t = weight_indices[t]

Sharding: column-parallel on vocab across 8 cores — weight and lora_B are
sharded along V (4000 rows/core), hidden / lora_A / weight_indices are
replicated.  Each core computes out[:, shard]; the host concatenates.

Per-core schedule (all fp32 data, fp32r matmuls):
  - consts, lora_A^T -> ATs scratch, lora_B^T -> BTs scratch
  - hT build tokens 0:512, expert mask, xa^T group 0
  - sub-pass A1 (tokens 0:512): FUSED weight transpose — per 512-wide
    vocab panel and k-tile, PE-transpose the natural weight block,
    matmul it immediately (4 PSUM banks accumulate, transposes cycle the
    other banks), and store the transposed panel to WTs DRAM scratch.
    hT for tokens 512:1024 is built concurrently (disjoint hT slots).
  - xa^T group 1; sub-pass A2 (tokens 512:1024): streams WTs, 4 banks.
  - hT rebuild + mask/xa for tokens 1024:2048
  - pass B (tokens 1024:2048): streams WTs, 8 PSUM banks.
"""

import numpy as np

T = 2048
H = 4096
V = 32000
NCORES = 8
VS = V // NCORES  # 4000
E = 8
R = 16
P = 128
KT = H // P  # 32
TB = 1024  # tokens per hT residency
PANELS = [(i * 512, 512) for i in range(7)] + [(3584, VS - 3584)]  # 7x512 + 416

_CACHE = {}


def _build_nc():
    from concourse import bacc
    import concourse.mybir as mybir
    from concourse.tile import TileContext
    from concourse.masks import make_identity

    f32 = mybir.dt.float32
    f32r = mybir.dt.float32r
    bf16 = mybir.dt.bfloat16
    i32 = mybir.dt.int32
    OP = mybir.AluOpType

    nc = bacc.Bacc("TRN2", target_bir_lowering=False, debug=False)

    hid_d = nc.dram_tensor("hidden", [T, H], f32, kind="ExternalInput")
    w_d = nc.dram_tensor("weight", [VS, H], f32, kind="ExternalInput")
    la_d = nc.dram_tensor("lora_a", [P, H], f32, kind="ExternalInput")  # [E*R, H]
    lb_d = nc.dram_tensor("lora_b", [E, VS, R], f32, kind="ExternalInput")
    idx_d = nc.dram_tensor("widx", [1, T], i32, kind="ExternalInput")
    out_d = nc.dram_tensor("out", [T, VS], f32, kind="ExternalOutput")

    wts_d = nc.dram_tensor("wts", [KT, P, VS], f32r, kind="Internal")
    ats_d = nc.dram_tensor("ats", [KT, P, P], f32r, kind="Internal")
    bts_d = nc.dram_tensor("bts", [P, VS], f32r, kind="Internal")

    with TileContext(nc) as tc:
        ident, free_ident = tc.tile([P, P], f32, name="ident")
        make_identity(nc, ident)
        hT, free_hT = tc.tile([P, KT * TB], f32r, name="hT")
        hT_k = hT.rearrange("p (k t) -> p k t", t=TB)

        with (
            tc.tile_pool(name="psp", bufs=8, space="PSUM") as psp,
            tc.tile_pool(name="natp", bufs=6) as natp,      # [128,512] f32 staging
            tc.tile_pool(name="wstp", bufs=2) as wstp,      # [128,512] f32r wT tiles
            tc.tile_pool(name="nathp", bufs=3) as nathp,    # [128,1024] f32 staging
            tc.tile_pool(name="wldp", bufs=2) as wldp,      # [128,2048] f32r loads
            tc.tile_pool(name="atldp", bufs=2) as atldp,    # [128,512] f32r loads
            tc.tile_pool(name="btldp", bufs=2) as btldp,    # [128,512] f32r loads
            tc.tile_pool(name="ostp", bufs=2) as ostp,      # [128,512] f32 out staging
            tc.tile_pool(name="maskp", bufs=1) as maskp,
            tc.tile_pool(name="btstp", bufs=1) as btstp,    # [16,512] f32r staging
        ):
            # ---- constants for mask ----
            p_col_i = maskp.tile([P, 1], i32, tag="pci")
            nc.gpsimd.iota(p_col_i, pattern=[[0, 1]], base=0, channel_multiplier=1)
            p_col = maskp.tile([P, 1], f32, tag="pcf")
            nc.vector.tensor_copy(p_col, p_col_i)

            # ---- lora_A^T -> ATs ----
            for q in range(4):
                nat_a = nathp.tile([P, 1024], f32, tag="nath")
                nc.sync.dma_start(nat_a, la_d[:, q * 1024:(q + 1) * 1024])
                for ks in range(2):
                    ps = psp.tile([P, 512], f32, tag="bank")
                    for kk in range(4):
                        nc.tensor.transpose(
                            ps[:, kk * P:(kk + 1) * P],
                            nat_a[:, (ks * 4 + kk) * P:(ks * 4 + kk + 1) * P],
                            ident,
                        )
                    st = wstp.tile([P, 512], f32r, tag="wst")
                    nc.vector.tensor_copy(st, ps)
                    k0 = q * 8 + ks * 4
                    nc.sync.dma_start(
                        ats_d[k0:k0 + 4, :, :].rearrange("k h e -> h k e"), st
                    )

            # ---- lora_B^T -> BTs ----
            NB_FULL = VS // P  # 31
            REM = VS % P       # 32
            for e in range(E):
                nat_b = natp.tile([P, 512], f32, tag="nat")
                nc.sync.dma_start(
                    nat_b[:, 0:NB_FULL * R],
                    lb_d[e, 0:NB_FULL * P, :].rearrange("(vt v) r -> v vt r", v=P),
                )
                nc.sync.dma_start(
                    nat_b[0:REM, NB_FULL * R:NB_FULL * R + R],
                    lb_d[e, NB_FULL * P:VS, :],
                )
                for grp in range(8):
                    vts = [grp * 4 + j for j in range(4)]
                    ps = psp.tile([P, 512], f32, tag="bank")
                    off = 0
                    for vt in vts:
                        vsz = P if vt < NB_FULL else REM
                        nc.tensor.transpose(
                            ps[0:R, off:off + vsz],
                            nat_b[0:vsz, vt * R:(vt + 1) * R],
                            ident[0:vsz, 0:vsz],
                        )
                        off += vsz
                    st = btstp.tile([R, 512], f32r, tag="btst")
                    nc.vector.tensor_copy(st[:, 0:off], ps[0:R, 0:off])
                    v0 = vts[0] * P
                    nc.sync.dma_start(
                        bts_d[e * R:(e + 1) * R, v0:v0 + off], st[:, 0:off]
                    )

            def build_hT(t0, tloc0, ntok):
                """Transpose hidden[t0:t0+ntok] into hT slots tloc0:tloc0+ntok."""
                for t8 in range(ntok // P):
                    for q in range(4):
                        nat_h = nathp.tile([P, 1024], f32, tag="nath")
                        nc.sync.dma_start(
                            nat_h,
                            hid_d[
                                t0 + t8 * P:t0 + (t8 + 1) * P,
                                q * 1024:(q + 1) * 1024,
                            ],
                        )
                        for ks in range(2):
                            ps = psp.tile([P, 512], f32, tag="bank")
                            for kk in range(4):
                                nc.tensor.transpose(
                                    ps[:, kk * P:(kk + 1) * P],
                                    nat_h[:, (ks * 4 + kk) * P:(ks * 4 + kk + 1) * P],
                                    ident,
                                )
                            k0 = q * 8 + ks * 4
                            nc.vector.tensor_copy(
                                hT_k[
                                    :, k0:k0 + 4,
                                    tloc0 + t8 * P:tloc0 + (t8 + 1) * P,
                                ],
                                ps,
                            )

            def build_mask(t0):
                """mask[p,t] = (widx[t0+t] == p//16) as bf16 [128, TB]."""
                idxp = maskp.tile([1, TB], i32, tag="idxp")
                nc.sync.dma_start(idxp, idx_d[:, t0:t0 + TB])
                idx_f = maskp.tile([1, TB], f32, tag="idxf")
                nc.vector.tensor_copy(idx_f, idxp)
                idx16 = maskp.tile([1, TB], bf16, tag="idx16")
                nc.vector.tensor_scalar_mul(idx16, idx_f, 16.0)
                bc = maskp.tile([P, TB], bf16, tag="bc")
                nc.gpsimd.partition_broadcast(bc, idx16)
                d = maskp.tile([P, TB], bf16, tag="d")
                nc.vector.tensor_scalar(d, bc, p_col, None, OP.subtract)
                u1 = maskp.tile([P, TB], bf16, tag="u1")
                nc.vector.tensor_scalar(u1, d, 0.0, None, OP.is_le)
                nc.vector.tensor_scalar(d, d, -15.0, None, OP.is_ge)
                mask = bc  # reuse slot: mask = u1 * d
                nc.vector.tensor_tensor(mask, u1, d, OP.mult)
                return mask

            def xa_group(g, mxaT, mask):
                """mxaT[:, g*512:(g+1)*512] = (A_all^T @ hT-group-g) * mask."""
                xa_ps = psp.tile([P, 512], f32, tag="bank")
                for kq in range(KT // 4):
                    atb = atldp.tile([P, 512], f32r, tag="atld")
                    nc.scalar.dma_start(
                        atb.rearrange("p (k e) -> p k e", e=P),
                        ats_d[4 * kq:4 * kq + 4, :, :].rearrange("k h e -> h k e"),
                    )
                    for kk in range(4):
                        k = 4 * kq + kk
                        nc.tensor.matmul(
                            xa_ps,
                            atb[:, kk * P:(kk + 1) * P],
                            hT_k[:, k, g * 512:(g + 1) * 512],
                            start=(k == 0),
                            stop=(k == KT - 1),
                        )
                nc.vector.tensor_tensor(
                    mxaT[:, g * 512:(g + 1) * 512],
                    xa_ps,
                    mask[:, g * 512:(g + 1) * 512],
                    OP.mult,
                )

            def finish_panel(accs, btk, mxaT, tloc0, t0, v0, np_):
                """lora matmul + copy-out + store for one panel."""
                nt = len(accs)
                for i in range(nt):
                    nc.tensor.matmul(
                        accs[i][:, 0:np_],
                        mxaT[:, tloc0 + i * P:tloc0 + (i + 1) * P],
                        btk[:, 0:np_],
                        start=False,
                        stop=True,
                    )
                for i in range(nt):
                    o_sb = ostp.tile([P, 512], f32, tag="ost")
                    nc.vector.tensor_copy(o_sb[:, 0:np_], accs[i][:, 0:np_])
                    nc.scalar.dma_start(
                        out_d[t0 + i * P:t0 + (i + 1) * P, v0:v0 + np_],
                        o_sb[:, 0:np_],
                    )

            def stream_panel(mxaT, tloc0, t0, ntiles, v0, np_, namepfx):
                """One vocab panel streaming WTs, ntiles token tiles."""
                accs = [
                    psp.tile([P, 512], f32, tag="bank", name=f"{namepfx}_{v0}_{i}")
                    for i in range(ntiles)
                ]
                btk = btldp.tile([P, 512], f32r, tag="btld")
                nc.scalar.dma_start(btk[:, 0:np_], bts_d[:, v0:v0 + np_])
                for kq in range(KT // 4):
                    wld = wldp.tile([P, 2048], f32r, tag="wld")
                    nc.scalar.dma_start(
                        wld.rearrange("p (k v) -> p k v", v=512)[:, :, 0:np_],
                        wts_d[4 * kq:4 * kq + 4, :, v0:v0 + np_].rearrange(
                            "k h v -> h k v"
                        ),
                    )
                    for kk in range(4):
                        k = 4 * kq + kk
                        for i in range(ntiles):
                            nc.tensor.matmul(
                                accs[i][:, 0:np_],
                                hT_k[:, k, tloc0 + i * P:tloc0 + (i + 1) * P],
                                wld[:, kk * 512:kk * 512 + np_],
                                start=(k == 0),
                                stop=False,
                            )
                finish_panel(accs, btk, mxaT, tloc0, t0, v0, np_)

            # ================= tokens 0:1024 =================
            build_hT(0, 0, 512)
            mask_a = build_mask(0)
            mxaT_a = maskp.tile([P, TB], f32r, tag="mxa")
            xa_group(0, mxaT_a, mask_a)

            # ---- A1 (tokens 0:512): fused W transpose + matmul + WTs store ----
            for pi, (v0, np_) in enumerate(PANELS):
                vbs = []  # (voff_local, vsz)
                off = 0
                while off < np_:
                    vsz = min(P, np_ - off)
                    vbs.append((off, vsz))
                    off += vsz
                accs = [
                    psp.tile([P, 512], f32, tag="bank", name=f"a1_{v0}_{i}")
                    for i in range(4)
                ]
                btk = btldp.tile([P, 512], f32r, tag="btld")
                nc.scalar.dma_start(btk[:, 0:np_], bts_d[:, v0:v0 + np_])
                for ks in range(8):
                    nats = []
                    for (vo, vsz) in vbs:
                        nat_w = natp.tile([P, 512], f32, tag="nat")
                        nc.sync.dma_start(
                            nat_w[0:vsz, :],
                            w_d[v0 + vo:v0 + vo + vsz, ks * 512:(ks + 1) * 512],
                        )
                        nats.append(nat_w)
                    for kk in range(4):
                        k = ks * 4 + kk
                        ps = psp.tile([P, 512], f32, tag="bank")
                        for (vo, vsz), nat_w in zip(vbs, nats):
                            nc.tensor.transpose(
                                ps[:, vo:vo + vsz],
                                nat_w[0:vsz, kk * P:(kk + 1) * P],
                                ident[0:vsz, 0:vsz],
                            )
                        wst = wstp.tile([P, 512], f32r, tag="wst")
                        nc.vector.tensor_copy(wst[:, 0:np_], ps[:, 0:np_])
                        nc.sync.dma_start(
                            wts_d[k, :, v0:v0 + np_], wst[:, 0:np_]
                        )
                        for i in range(4):
                            nc.tensor.matmul(
                                accs[i][:, 0:np_],
                                hT_k[:, k, i * P:(i + 1) * P],
                                wst[:, 0:np_],
                                start=(k == 0),
                                stop=False,
                            )
                finish_panel(accs, btk, mxaT_a, 0, 0, v0, np_)
                if pi == 2:
                    # overlap: build hT for tokens 512:1024 during A1's tail
                    # (disjoint hT slots — no conflict with A1 reads)
                    build_hT(512, 512, 512)

            # ---- A2 (tokens 512:1024): stream WTs ----
            xa_group(1, mxaT_a, mask_a)
            for (v0, np_) in PANELS:
                stream_panel(mxaT_a, 512, 512, 4, v0, np_, "a2")

            # ================= tokens 1024:2048 =================
            build_hT(TB, 0, TB)
            mask_b = build_mask(TB)
            mxaT_b = maskp.tile([P, TB], f32r, tag="mxa", name="mxa_b")
            xa_group(0, mxaT_b, mask_b)
            xa_group(1, mxaT_b, mask_b)

            for (v0, np_) in PANELS:
                stream_panel(mxaT_b, 0, TB, 8, v0, np_, "b")

        free_hT()
        free_ident()

    nc.finalize()
    return nc


def _get_nc():
    if "nc" not in _CACHE:
        _CACHE["nc"] = _build_nc()
    return _CACHE["nc"]


def run_sharded(inputs, trace=False):
    from concourse import bass_utils

    hidden = np.ascontiguousarray(inputs["hidden_states"], dtype=np.float32)
    weight = np.ascontiguousarray(inputs["weight"], dtype=np.float32)
    lora_A = np.ascontiguousarray(inputs["lora_A"], dtype=np.float32).reshape(E * R, H)
    lora_B = np.ascontiguousarray(inputs["lora_B"], dtype=np.float32)
    widx = np.ascontiguousarray(inputs["weight_indices"], dtype=np.int32).reshape(1, T)

    nc = _get_nc()
    in_maps = []
    for c in range(NCORES):
        in_maps.append(
            {
                "hidden": hidden,
                "weight": weight[c * VS:(c + 1) * VS],
                "lora_a": lora_A,
                "lora_b": lora_B[:, c * VS:(c + 1) * VS, :],
                "widx": widx,
            }
        )
    res = bass_utils.run_bass_kernel_spmd(
        nc, in_maps, core_ids=list(range(NCORES)), trace=trace
    )
    out = np.concatenate([res.results[c]["out"] for c in range(NCORES)], axis=1)
    return out, res


def kernel(**inputs) -> np.ndarray:
    out, _ = run_sharded(inputs, trace=False)
    return out
